# revision 32
# baseline (speedup 1.0000x reference)
"""Trainium2 Bass kernel for nn_ClassifierGuided (2-modality top-12-of-16 MoE classifier).

Sharding: pure data-parallel over tokens. 2 modalities x 4096 tokens; each of
the 8 cores owns 1024 tokens of one modality (cores 0-3 -> modality 0, cores
4-7 -> modality 1) and that modality's full weights. Dense-eval MoE (all 16
experts computed, sparse gates applied), so no all-to-all is needed.

Expert matmuls run in fp8 (e4m3) with the PE DoubleRow perf mode: each matmul
contracts 256 k-dims (2 packed rows per partition) at 0.5 cycles per output
row -- 4x the fp32r rate. Scale scheme keeps every fused op cheap:
  xsb   = 512*x   (bf16; gating/residual/head path; Wg, Wo divided by 512)
  x8    = x       (fp8 moving operand of W1)
  w18   = 64*W1, w28 = 64*W2, gates cast to fp8 as 8*g, b28 = 512*b2/8
  h_psum = 64*(x@W1) -> ACT relu(in/64 + b1) -> h parked in SBUF (fp32)
  hg8   = h * (8g) in fp8 (DVE/Pool, fused multiply + cast)
  moe_psum = hg8@(64*W2) + (8g)@(512*b2/8) = 512*(moe + b2 term)
  drain: z' = max(moe_psum,0) + xsb = 512*z (single fused op), head uses Wo/512.

Parking h in SBUF frees each h-PSUM bank right after the relu, so the
gate-broadcast DMA latency never blocks the W1 pipeline.
"""
import sys

sys.path.insert(0, "/opt/trn_rl_repo")

import ml_dtypes
import numpy as np

import concourse.bass as bass
import concourse.mybir as mybir
import concourse.tile as tile
from concourse import bacc
from concourse.bass_utils import run_bass_kernel_spmd
from concourse.masks import make_identity

# ---- problem sizes (hardcoded per the harness contract) ----
B = 4096           # tokens per modality
D = 768            # model dim
E = 16             # experts
H = 192            # expert hidden
O = 101            # classifier out
KTOP = 12          # top-k experts
NCORES = 8
BC = B // 4        # 1024 tokens per core
DC = D // 128      # 6 d-chunks
NT = 512           # token tile (matmul moving dim)
NTILES = BC // NT  # 2
NHC = E * H // 128 # 24 h-chunks of 128
NK1 = D // 256     # 3 DoubleRow k-tiles for W1 (contract d)
NK2 = E * H // 256 # 12 DoubleRow k-tiles for W2 (contract h)
NPAIR = E // 2     # 8 expert pairs (one gb broadcast tile each)
F32 = mybir.dt.float32
BF16 = mybir.dt.bfloat16
F8 = mybir.dt.float8e4
DR = mybir.MatmulPerfMode.DoubleRow
NEG_BIG = -1.0e30

SX = 512.0   # x residual scale (Wg, Wo pre-divided on host)
SW = 64.0    # fp8 weight scale for W1/W2
SG = 8.0     # fp8 gate scale  (SW * SG == SX keeps the drain exact)

_NC_CACHE = {}


def build_nc():
    nc = bacc.Bacc("TRN2", target_bir_lowering=False, debug=False,
                   num_devices=NCORES)

    # ---- DRAM I/O (per-core views; host pre-packs) ----
    xT = nc.dram_tensor("xT", [D, BC], BF16, kind="ExternalInput").ap()       # 512*x
    x8d = nc.dram_tensor("x8d", [128, 2, NK1, BC], F8, kind="ExternalInput").ap()
    w18d = nc.dram_tensor("w18d", [128, 2, NK1, E * H], F8, kind="ExternalInput").ap()
    w28d = nc.dram_tensor("w28d", [128, 2, NK2, D], F8, kind="ExternalInput").ap()
    b1p = nc.dram_tensor("b1p", [128, NHC], F32, kind="ExternalInput").ap()
    b28d = nc.dram_tensor("b28d", [8, 2, D], F8, kind="ExternalInput").ap()
    wg = nc.dram_tensor("wg", [D, E], BF16, kind="ExternalInput").ap()        # Wg/512
    wo = nc.dram_tensor("wo", [D, O], BF16, kind="ExternalInput").ap()        # Wo/512
    bo = nc.dram_tensor("bo", [1, O], BF16, kind="ExternalInput").ap()
    outT = nc.dram_tensor("outT", [O, BC], F32, kind="ExternalOutput").ap()

    with tile.TileContext(nc) as tc:
        with tc.tile_pool(name="const", bufs=1) as cpool:
            # resident SBUF tensors
            xsb = cpool.tile([128, DC, BC], BF16)            # 512*xT; later 512*zT
            x8 = cpool.tile([128, 2, NK1, BC], F8)
            w18 = cpool.tile([128, 2, NK1, E * H], F8)
            w28 = cpool.tile([128, 2, NK2, D], F8)
            b1sb = cpool.tile([128, NHC], F32)
            b28 = cpool.tile([8, 2, D], F8)
            wgf = cpool.tile([128, DC, E], BF16)
            wosb = cpool.tile([128, DC, O], BF16)
            bosb = cpool.tile([1, O], BF16)
            ones = cpool.tile([1, NT], BF16)
            nc.gpsimd.memset(ones[:, :], 1.0)
            gT = cpool.tile([E, BC], F32)                    # gates, expert-major
            gT8 = cpool.tile([E, BC], F8)                    # 8*gates, fp8
            gTa8 = cpool.tile([8, 2, BC], F8)                # 8*gates, DoubleRow view
            ident = cpool.tile([128, 128], F32)

            make_identity(nc, ident[:, :])

            # ---- loads, per-queue FIFO in need order ----
            # SP: gating weights + half of x (gating-critical), then the gate
            # flush/broadcast DMAs (emitted inline) and the output writes.
            # ACT: other half of x (ACT compute starts only at first relu).
            # Pool: fp8 x + expert weights, interleaved in use order.
            nc.sync.dma_start(out=wgf[:, :, :],
                              in_=wg.rearrange("(c p) e -> p c e", p=128))
            for c in (0, 2, 4):
                nc.sync.dma_start(out=xsb[:, c, :], in_=xT[128 * c:128 * (c + 1), :])
            nc.sync.dma_start(out=b1sb[:, :], in_=b1p[:, :])
            for c in (1, 3, 5):
                nc.scalar.dma_start(out=xsb[:, c, :], in_=xT[128 * c:128 * (c + 1), :])
            # Pool, in use order: x8 + first W1 columns, then W2 k-tiles
            # interleaved with later W1 columns so both streams stay ahead.
            for kt in range(NK1):
                nc.gpsimd.dma_start(out=x8[:, :, kt, :], in_=x8d[:, :, kt, :])
            HQ = E * H // 4
            for kt in range(NK1):
                nc.gpsimd.dma_start(out=w18[:, :, kt, 0:HQ],
                                    in_=w18d[:, :, kt, 0:HQ])
            for g2 in range(3):
                nc.gpsimd.dma_start(out=w28[:, :, 2 * g2:2 * g2 + 2, :],
                                    in_=w28d[:, :, 2 * g2:2 * g2 + 2, :])
                for kt in range(NK1):
                    nc.gpsimd.dma_start(
                        out=w18[:, :, kt, HQ * (g2 + 1):HQ * (g2 + 2)],
                        in_=w18d[:, :, kt, HQ * (g2 + 1):HQ * (g2 + 2)])
            for g2 in range(3, 6):
                nc.gpsimd.dma_start(out=w28[:, :, 2 * g2:2 * g2 + 2, :],
                                    in_=w28d[:, :, 2 * g2:2 * g2 + 2, :])
            nc.gpsimd.dma_start(out=b28[:, :, :], in_=b28d)
            nc.gpsimd.dma_start(out=wosb[:, :, :],
                                in_=wo.rearrange("(c p) o -> p c o", p=128))
            nc.gpsimd.dma_start(out=bosb[:, :], in_=bo[:, :])

            # gates round-trip through DRAM (fp8); gate-broadcast tiles are
            # read back with partition-step-0 DMAs
            gdram8 = cpool.tile([E, BC], F8, space="DRAM")
            gb_tiles = {}
            gbmix = {}
            ctx_gb = tc.tile_pool(name="gbpool", bufs=2 * NPAIR + 2)
            gbpool = ctx_gb.__enter__()

            def load_gb(t, p):
                # gb[:,0,:] = 8*gate(e0) broadcast over partitions; [:,1,:] = e1
                gb = gbpool.tile([128, 2, NT], F8, tag="gb", name="gb")
                gb_src = bass.AP(tensor=gdram8.tensor,
                                 offset=2 * p * BC + NT * t,
                                 ap=[[0, 128], [BC, 2], [1, NT]])
                nc.sync.dma_start(out=gb[:, :, :], in_=gb_src)
                gb_tiles[(t, p)] = gb

            def load_gbmix_group(t, q):
                # mixed-chunk gate tile, pair group q (pairs 2q, 2q+1): for
                # pair p, partitions 0:64 carry e(2p) and 64:128 carry
                # e(2p+1) -- matches the partition split of the middle
                # h-chunk, so its gate-mult is one op
                gm = gbmix[t]
                for half in range(2):
                    src = bass.AP(tensor=gdram8.tensor,
                                  offset=half * BC + 4 * q * BC + NT * t,
                                  ap=[[0, 64], [2 * BC, 2], [1, NT]])
                    nc.sync.dma_start(
                        out=gm[64 * half:64 * (half + 1), 2 * q:2 * q + 2, :],
                        in_=src)

            def flush_gates(t):
                ts = slice(NT * t, NT * (t + 1))
                # cast fp32 gates -> 8*g in fp8, then flush for broadcast reads
                nc.vector.tensor_scalar_mul(gT8[:, ts], gT[:, ts], SG)
                nc.sync.dma_start(out=gdram8[:, ts], in_=gT8[:, ts])
                # DoubleRow-packed gate view for the b2 bias matmul
                ga_src = bass.AP(tensor=gdram8.tensor, offset=NT * t,
                                 ap=[[BC, 8], [8 * BC, 2], [1, NT]])
                nc.sync.dma_start(out=gTa8[:, :, ts], in_=ga_src)
                gbmix[t] = gbpool.tile([128, NPAIR, NT], F8, tag="gbm",
                                       name="gbm")
                # interleave pure-pair and mixed-group loads in consumer order
                load_gb(t, 0)
                load_gbmix_group(t, 0)
                load_gb(t, 1)
                for q in range(1, NPAIR // 2):
                    load_gb(t, 2 * q)
                    load_gbmix_group(t, q)
                    load_gb(t, 2 * q + 1)

            # ---------------- main-loop machinery ----------------
            ctx_hps = tc.tile_pool(name="hps", bufs=2, space="PSUM")
            hps = ctx_hps.__enter__()
            ctx_hpark = tc.tile_pool(name="hpark", bufs=10)
            hpark_pool = ctx_hpark.__enter__()
            hpark = {}

            def stage1a(t, hc):
                # W1 (fp8 DoubleRow) + relu; h parked in SBUF, PSUM bank freed
                ts = slice(NT * t, NT * (t + 1))
                hps_t = hps.tile([128, NT], F32, tag="h")
                for kt in range(NK1):
                    nc.tensor.matmul(hps_t[:, :],
                                     w18[:, :, kt, 128 * hc:128 * (hc + 1)],
                                     x8[:, :, kt, ts],
                                     start=(kt == 0), stop=(kt == NK1 - 1),
                                     perf_mode=DR)
                hp = hpark_pool.tile([128, NT], F32, tag="hp", name="hp")
                nc.scalar.activation(hp[:, :], hps_t[:, :],
                                     mybir.ActivationFunctionType.Relu,
                                     bias=b1sb[:, hc:hc + 1], scale=1.0 / SW)
                hpark[(t, hc)] = hp

            # ---------------- gating pass (128-token subtiles) ----------------
            # W1 for the first few h-chunks is emitted first: it only needs
            # x8/w18, so the PE warms up while the bf16 x chunks still stream.
            for hc in range(8):
                stage1a(0, hc)

            with tc.tile_pool(name="gps", bufs=2, space="PSUM") as gps, \
                 tc.tile_pool(name="gtp", bufs=2, space="PSUM") as gtp, \
                 tc.tile_pool(name="gsb", bufs=3) as gsb:
                for i in range(BC // 128):
                    if i * 128 % NT == 0 and i > 0:
                        flush_gates(i * 128 // NT - 1)
                    ts = slice(128 * i, 128 * (i + 1))
                    lg_ps = gps.tile([128, E], F32, tag="lg")
                    for c in range(DC):
                        nc.tensor.matmul(lg_ps[:, :], xsb[:, c, ts],
                                         wgf[:, c, :],
                                         start=(c == 0), stop=(c == DC - 1))
                    lg = gsb.tile([128, E], F32, tag="lg_sb")
                    nc.vector.tensor_copy(lg[:, :], lg_ps[:, :])
                    # top-8 values, then values 9..16 after masking them out
                    t8a = gsb.tile([128, 8], F32, tag="t8a")
                    nc.vector.max(t8a[:, :], lg[:, :])
                    l2 = gsb.tile([128, E], F32, tag="l2")
                    nc.vector.match_replace(l2[:, :], t8a[:, :], lg[:, :], NEG_BIG)
                    t8b = gsb.tile([128, 8], F32, tag="t8b")
                    nc.vector.max(t8b[:, :], l2[:, :])
                    # softmax over entries >= 12th-largest (t8b[:,3])
                    e16 = gsb.tile([128, E], F32, tag="e16")
                    nc.scalar.activation(e16[:, :], lg[:, :],
                                         mybir.ActivationFunctionType.Exp)
                    em = gsb.tile([128, E], F32, tag="em")
                    ssum = gsb.tile([128, 1], F32, tag="ssum")
                    nc.vector.scalar_tensor_tensor(
                        out=em[:, :], in0=lg[:, :], scalar=t8b[:, 3:4],
                        in1=e16[:, :], op0=mybir.AluOpType.is_ge,
                        op1=mybir.AluOpType.mult, accum_out=ssum[:, :])
                    rinv = gsb.tile([128, 1], F32, tag="rinv")
                    nc.vector.reciprocal(rinv[:, :], ssum[:, :])
                    g = gsb.tile([128, E], F32, tag="g")
                    nc.vector.tensor_scalar_mul(g[:, :], em[:, :], rinv[:, :])
                    # transpose to expert-major gT[16, tokens]
                    gt_ps = gtp.tile([E, 128], F32, tag="gt")
                    nc.tensor.transpose(gt_ps[:, :], g[:, :], ident[:, :])
                    nc.vector.tensor_copy(gT[:, ts], gt_ps[:, :])

            flush_gates(NTILES - 1)

            # ---------------- main loop ----------------
            with tc.tile_pool(name="moeps", bufs=DC, space="PSUM") as moeps, \
                 tc.tile_pool(name="hgpool", bufs=2) as hgpool, \
                 tc.tile_pool(name="opool", bufs=4) as opool:
                for t in range(NTILES):
                    ts = slice(NT * t, NT * (t + 1))
                    moe = [moeps.tile([128, NT], F32, tag="moe", name="moe")
                           for _ in range(DC)]
                    hg8 = hgpool.tile([128, 2, NK2, NT], F8, tag="hg", name="hg")

                    def mult_chunk(hc, t=t, hg8=hg8):
                        # hg8 = h * (8g): fused multiply + fp8 cast from the
                        # parked h. Early tile 0 runs on DVE (Pool is still
                        # streaming weight DMAs); later chunks shift to Pool.
                        p, m = divmod(hc, 3)
                        if t == 0:
                            eng = nc.vector if hc < 16 else nc.gpsimd
                        else:
                            eng = nc.vector if m == 0 else nc.gpsimd
                        if m == 1:
                            gbs = gbmix[t][:, p, :]
                        else:
                            gbs = gb_tiles[(t, p)][:, 0 if m == 0 else 1, :]
                        hp = hpark.pop((t, hc))
                        eng.tensor_tensor(out=hg8[:, hc % 2, hc // 2, :],
                                          in0=hp[:, :], in1=gbs,
                                          op=mybir.AluOpType.mult)

                    # head runs as two token-halves (384/128) in PSUM tiles
                    # reclaimed from the moe pool; bo is folded in as a
                    # ones-row matmul and outT DMAs read PSUM directly, so
                    # the tail is just the short half's close + DMA
                    HSPL = (slice(0, 384), slice(384, NT))
                    head_ps = [None, None]

                    def head_chunk(c, halves=(0, 1), t=t):
                        for hf in halves:
                            if head_ps[hf] is None:
                                head_ps[hf] = moeps.tile(
                                    [O, HSPL[hf].stop - HSPL[hf].start], F32,
                                    tag="moe", name="head")
                            nc.tensor.matmul(
                                head_ps[hf][:, :], wosb[:, c, :],
                                xsb[:, c, NT * t + HSPL[hf].start:
                                    NT * t + HSPL[hf].stop],
                                start=(c == 0), stop=False)

                    def head_close(hf, t=t):
                        hs = HSPL[hf]
                        nc.tensor.matmul(head_ps[hf][:, :], bosb[:, :],
                                         ones[:, hs], start=False,
                                         stop=True)
                        osb = opool.tile([O, hs.stop - hs.start], F32,
                                         tag="osb", name="osb")
                        nc.vector.tensor_copy(osb[:, :], head_ps[hf][:, :])
                        nc.sync.dma_start(
                            out=outT[:, NT * t + hs.start:NT * t + hs.stop],
                            in_=osb[:, :])

                    def finish_chunk(c, moe=moe, ts=ts, t=t):
                        # z' = max(moe_psum,0) + 512x in one fused op, in place
                        eng = nc.vector if t == 0 else nc.gpsimd
                        eng.scalar_tensor_tensor(
                            out=xsb[:, c, ts], in0=moe[c][:, :], scalar=0.0,
                            in1=xsb[:, c, ts],
                            op0=mybir.AluOpType.max, op1=mybir.AluOpType.add)

                    def stage2_ktile(k, close, moe=moe, hg8=hg8, ts=ts):
                        for c in range(DC):
                            nc.tensor.matmul(moe[c][:, :],
                                             w28[:, :, k, 128 * c:128 * (c + 1)],
                                             hg8[:, :, k, :],
                                             start=(k == 0), stop=False,
                                             perf_mode=DR)
                            if close:
                                # b2 bias term closes this chunk's accumulation
                                nc.tensor.matmul(moe[c][:, :],
                                                 b28[:, :, 128 * c:128 * (c + 1)],
                                                 gTa8[:, :, ts],
                                                 start=False, stop=True,
                                                 perf_mode=DR)
                                finish_chunk(c)
                                # head matmul trails two chunks behind so its
                                # relu+residual drain is already complete
                                if c >= 2:
                                    head_chunk(c - 2)
                        if close:
                            # finish half A fully so its output DMA's init
                            # latency hides under half B's last matmuls
                            head_chunk(DC - 2, halves=(0,))
                            head_chunk(DC - 1, halves=(0,))
                            head_close(0)
                            head_chunk(DC - 2, halves=(1,))
                            head_chunk(DC - 1, halves=(1,))
                            head_close(1)

                    # software pipeline: W1+relu run 2-3 k-tiles ahead; the
                    # gate-mults are emitted just before their consumer
                    lead = 8 if t == 0 else 6
                    for k in range(NK2):
                        if t == 1 and k == 0:
                            for hc in range(6):
                                stage1a(t, hc)
                        for hc in range(2 * k + lead, min(2 * k + lead + 2, NHC)):
                            stage1a(t, hc)
                        mult_chunk(2 * k)
                        mult_chunk(2 * k + 1)
                        stage2_ktile(k, close=(k == NK2 - 1))
            ctx_hpark.__exit__(None, None, None)
            ctx_hps.__exit__(None, None, None)
            ctx_gb.__exit__(None, None, None)

    nc.compile()
    return nc


def _pack_core_inputs(x, Wg, W1, b1, W2, b2, Wo, bo, c4):
    """Per-core input dict for one modality's weights + 1024-token slice."""
    f = np.float32
    f8 = ml_dtypes.float8_e4m3
    bf = ml_dtypes.bfloat16
    tok = slice(BC * c4, BC * (c4 + 1))
    x = np.asarray(x[tok], f)                                    # [BC, 768]
    xT = np.ascontiguousarray(x.T)                               # [768, BC]
    # DoubleRow pack: k = 256*kt + 128*i + p  ->  [p, i, kt, ...]
    x8 = np.ascontiguousarray(
        xT.reshape(NK1, 2, 128, BC).transpose(2, 1, 0, 3)).astype(f8)
    W1r = np.asarray(W1, f).transpose(1, 0, 2).reshape(D, E * H)  # [768, 3072]
    w18 = np.ascontiguousarray(
        (SW * W1r).reshape(NK1, 2, 128, E * H).transpose(2, 1, 0, 3)).astype(f8)
    W2r = np.asarray(W2, f).reshape(E * H, D)                     # [3072, 768]
    w28 = np.ascontiguousarray(
        (SW * W2r).reshape(NK2, 2, 128, D).transpose(2, 1, 0, 3)).astype(f8)
    # b2 DoubleRow pack: expert e = p + 8*i; scaled so (8g)@b28 = 512*(g@b2)
    b28 = np.ascontiguousarray(
        (SX / SG) * np.asarray(b2, f).reshape(2, 8, D).transpose(1, 0, 2)).astype(f8)
    return {
        "xT": np.ascontiguousarray(SX * xT).astype(bf),
        "x8d": x8,
        "w18d": w18,
        "w28d": w28,
        "b1p": np.ascontiguousarray(np.asarray(b1, f).reshape(NHC, 128).T),
        "b28d": b28,
        "wg": (np.asarray(Wg, f) / SX).astype(bf),
        "wo": (np.asarray(Wo, f) / SX).astype(bf),
        "bo": np.ascontiguousarray(np.asarray(bo, f).reshape(1, O)).astype(bf),
    }


def run_on_hw(inputs, trace=False, **kw):
    if "nc" not in _NC_CACHE:
        _NC_CACHE["nc"] = build_nc()
    nc = _NC_CACHE["nc"]
    in_maps = []
    for core in range(NCORES):
        i, c4 = divmod(core, 4)
        x = inputs["x0"] if i == 0 else inputs["x1"]
        in_maps.append(_pack_core_inputs(
            x, inputs["Wg"][i], inputs["W1"][i], inputs["b1"][i],
            inputs["W2"][i], inputs["b2"][i], inputs["Wo"][i], inputs["bo"][i], c4))
    res = run_bass_kernel_spmd(nc, in_maps, core_ids=list(range(NCORES)),
                               trace=trace, **kw)
    outs = []
    for i in range(2):
        outs.append(np.concatenate(
            [res.results[4 * i + c]["outT"].T for c in range(4)], axis=0))
    return (outs[0], outs[1]), res


def kernel(**inputs):
    (o0, o1), _ = run_on_hw(inputs)
    return (o0, o1)
